# revision 9
# baseline (speedup 1.0000x reference)
"""HNet energy kernel v5: algebraic reduction to ONE fp8-DR GEMM stream.

Math (exact): es one-hot indicators decompose over the 2-bit temp
alphabet: A_t(a0,a1) = c_t + l0_t*a0 + l1_t*a1 + q_t*a0*a1 with
q_t in {+1,-1}.  Summing over kept edge codes v:

  energies[i,j] = const[j] + sum_n na[i,n] * W[j,n]
                 + sum_e AND[i,e] * Q[j,e]

with AND[i,e] = na[i,n0[e]] & na[i,n1[e]], Q in {-1,0,1} (disjoint
one-hots), W integer (scatter-add of per-endpoint counts; split into
fp8-exact parts with |w|<=16 if needed), const[j] = null_count +
sum c_t cnt_v.  That's 8192+1024 contraction instead of 2x8192:
576 DR matmuls/core instead of 1024, with ZERO on-device masking
(all operands host-built exact fp8).

Per core (4 point-groups x 2 cmp-groups): lhs XT ([128, 72*512] fp8,
36KB/partition) is RESIDENT in SBUF, DMA'd blockwise during pass 0 and
reused in pass 1.  rhs R blocks stream through a pool, DMA gated by
matmul consumption (pacing: unpaced bursts trip the board DMA throttle
-> PE locked at ~0.83x clock, measured v2/v3).  Pass 1 runs its two
cmp-tiles as sequential phases over resident rhs tiles so the first
half of the final output drains (copy+DMA wire time) under the last
~31us of compute; only 4 banks drain after the last matmul.
"""

import numpy as np
import ml_dtypes

import concourse.bacc as bacc
import concourse.mybir as mybir
from concourse.tile import TileContext
from concourse.bass_utils import run_bass_kernel_spmd

# ---- problem constants (hardcoded from spec) ----
N_PTS, N_NODES, N_EDGES, N_CMP = 2048, 1024, 8192, 4096
PGROUPS, CGROUPS = 4, 2          # 8 cores = 4 point-groups x 2 cmp-groups
P = N_PTS // PGROUPS             # 512 points per core
C = N_CMP // CGROUPS             # 2048 cmp columns per core
ECHUNKS = N_EDGES // 128         # 64 edge chunks of 128
NCHUNKS = N_NODES // 128         # 8 node chunks of 128 per W part
NTILES = C // 512                # 4 cmp tiles of 512 per core
MTILES = P // 128                # 4 point chunks of 128 per core
ABLK = 4                         # contraction chunks per streamed block
NPASS = 2                        # output passes (2 cmp tiles each)

FP8 = mybir.dt.float8e4
F32 = mybir.dt.float32
NP_FP8 = ml_dtypes.float8_e4m3
DR = mybir.MatmulPerfMode.DoubleRow

_CODE2TEMP = {2: 0, 3: 1, 5: 2, 9: 3}   # EDG code value -> temp index
# A_t(a0,a1) = c + l0*a0 + l1*a1 + q*a0*a1
_COEF = {0: (1, -1, -1, 1), 1: (0, 0, 1, -1),
         2: (0, 1, 0, -1), 3: (0, 0, 0, 1)}

_nc_cache: dict = {}


def _build_nc(nparts):
    """SPMD Bass program.  Contraction = 64 edge chunks + 8*nparts node
    chunks, all fp8 DoubleRow, 8 PSUM banks per pass, 2 passes."""
    nchunk = ECHUNKS + NCHUNKS * nparts
    nblks = nchunk // ABLK
    nc = bacc.Bacc(None)
    #   XT : [128, nchunk*P]          [ki, c*P+p] = X[pg*P+p, c*128+ki]
    #   RT : [NTILES, nblks, 128, ABLK*512]
    #        [nt, blk, ki, c*512+j] = R[cg*C+nt*512+j, (blk*ABLK+c)*128+ki]
    XT = nc.dram_tensor("XT", [128, nchunk * P], FP8, kind="ExternalInput")
    RT = nc.dram_tensor("RT", [NTILES, nblks, 128, ABLK * 512], FP8,
                        kind="ExternalInput")
    en = nc.dram_tensor("en", [P, C], F32, kind="ExternalOutput")

    with TileContext(nc) as tc:
        with (
            tc.tile_pool(name="const", bufs=1) as const_pool,
            tc.tile_pool(name="x", bufs=1) as x_pool,
            tc.tile_pool(name="rt", bufs=18) as rt_pool,
            tc.tile_pool(name="out", bufs=8) as out_pool,
            tc.tile_pool(name="psum", bufs=8, space="PSUM") as psum_pool,
        ):
            # PE clock warmup FIRST: dummy matmuls in the otherwise-idle
            # preamble window so the HAM ramp to full clock happens
            # before real data lands
            dummy = const_pool.tile([128, 2, 512], FP8, tag="dummy")
            nc.any.memset(dummy[:], 0.0)
            wps = psum_pool.tile([128, 512], F32, name="wps", tag="ps")
            for w in range(7):
                nc.tensor.matmul(wps, lhsT=dummy[:, :, 0:128], rhs=dummy[:],
                                 start=(w == 0), stop=(w == 6), perf_mode=DR)
            # ACT table prewarm (first ACTIVATE otherwise pays ~1.3us
            # table load when the mid-run drain copies start)
            warm = const_pool.tile([128, 1], F32, tag="warm")
            nc.any.memset(warm[:], 0.0)
            nc.scalar.copy(out=warm[:], in_=warm[:])

            xt = x_pool.tile([128, nchunk, P], FP8, tag="x")

            def drain(banks, final=False):
                """banks: list of (nt, m, ps) in completion order.  Only
                DVE (vector) and ACT (scalar) can read PSUM; mid-run the
                out DMAs go on the sync + gpsimd queues so the copy
                engines' chains are not interrupted by ~0.6us DMA
                issues.  The FINAL drain's wire time is the exposed
                tail: spread its 4 DMAs over 4 idle queues."""
                cengs = (nc.vector, nc.scalar)
                dengs = ((nc.sync, nc.gpsimd, nc.scalar, nc.sync)
                         if final else (nc.sync, nc.gpsimd))
                for k, (nt, m, ps) in enumerate(banks):
                    ot = out_pool.tile([128, 512], F32, name="ot", tag="out")
                    ceng = cengs[k % len(cengs)]
                    if ceng is nc.scalar:
                        nc.scalar.copy(out=ot[:], in_=ps[:])
                    else:
                        ceng.tensor_copy(ot[:], ps[:])
                    dengs[k % len(dengs)].dma_start(
                        out=en[m * 128:(m + 1) * 128,
                               nt * 512:(nt + 1) * 512],
                        in_=ot[:])

            # ---- pass 0: cmp tiles 0,1 interleaved; xt streams in ----
            # rhs DMAs alternate sync/gpsimd: one queue's wire rate
            # (~190 GB/s) runs at ~78% carrying the whole rt stream and
            # slips ~0.4us every ~3 blocks; two queues have slack
            eps0 = [[psum_pool.tile([128, 512], F32, name="ep", tag="ps")
                     for _m in range(MTILES)] for _ntl in range(2)]
            for blk in range(nblks):
                base = blk * ABLK
                rts = [rt_pool.tile([128, ABLK, 512], FP8, name="rt",
                                    tag="rt") for _ntl in range(2)]
                if blk == 0:
                    # startup critical path: first chunk-pair of XT and
                    # RT[0,0] land first, in halves, on separate queues
                    nc.scalar.dma_start(out=xt[:, 0:2, :],
                                        in_=XT[:, 0:2 * P])
                    nc.sync.dma_start(out=rts[0][:, 0:2, :],
                                      in_=RT[0, 0, :, 0:1024])
                    nc.scalar.dma_start(out=xt[:, 2:4, :],
                                        in_=XT[:, 2 * P:4 * P])
                    nc.gpsimd.dma_start(out=rts[0][:, 2:4, :],
                                        in_=RT[0, 0, :, 1024:2048])
                    nc.gpsimd.dma_start(out=rts[1][:], in_=RT[1, 0])
                else:
                    nc.scalar.dma_start(
                        out=xt[:, base:base + ABLK, :],
                        in_=XT[:, base * P:(base + ABLK) * P])
                    nc.sync.dma_start(out=rts[0][:], in_=RT[0, blk])
                    nc.gpsimd.dma_start(out=rts[1][:], in_=RT[1, blk])
                for cp in range(ABLK // 2):
                    first = (blk == 0 and cp == 0)
                    last = (blk == nblks - 1 and cp == ABLK // 2 - 1)
                    if first:
                        # rts[1] arrives later: run all ntl0 work first
                        order = [(ntl, m) for ntl in range(2)
                                 for m in range(MTILES)]
                    elif last:
                        # bank-major so each bank's stop lands early and
                        # drain copies pipeline with the last matmuls
                        order = [(ntl, m) for ntl in range(2)
                                 for m in range(MTILES)]
                    else:
                        # m-outer: consecutive ntl pair shares lhsT
                        order = [(ntl, m) for m in range(MTILES)
                                 for ntl in range(2)]
                    for ntl, m in order:
                        nc.tensor.matmul(
                            eps0[ntl][m],
                            lhsT=xt[:, base + 2 * cp:base + 2 * cp + 2,
                                    m * 128:(m + 1) * 128],
                            rhs=rts[ntl][:, 2 * cp:2 * cp + 2, :],
                            start=first, stop=last, perf_mode=DR)

            # ---- pass 1: cmp tiles 2,3 as sequential phases over ----
            # ---- resident rhs tiles; xt already resident           ----
            eps1 = [[psum_pool.tile([128, 512], F32, name="ep", tag="ps")
                     for _m in range(MTILES)] for _ntl in range(2)]
            drain([(ntl, m, eps0[ntl][m])
                   for ntl in range(2) for m in range(MTILES)])
            for ntl in range(2):
                for blk in range(nblks):
                    base = blk * ABLK
                    rt = rt_pool.tile([128, ABLK, 512], FP8, name="rt",
                                      tag="rt")
                    deng = nc.sync if blk % 2 == 0 else nc.gpsimd
                    deng.dma_start(out=rt[:], in_=RT[2 + ntl, blk])
                    for cp in range(ABLK // 2):
                        first = (blk == 0 and cp == 0)
                        last = (blk == nblks - 1 and cp == ABLK // 2 - 1)
                        for m in range(MTILES):
                            nc.tensor.matmul(
                                eps1[ntl][m],
                                lhsT=xt[:, base + 2 * cp:base + 2 * cp + 2,
                                        m * 128:(m + 1) * 128],
                                rhs=rt[:, 2 * cp:2 * cp + 2, :],
                                start=first, stop=last, perf_mode=DR)
                if ntl == 0:
                    # phase 0 banks drain under phase 1's ~31us compute
                    drain([(2, m, eps1[0][m]) for m in range(MTILES)])
            drain([(3, m, eps1[1][m]) for m in range(MTILES)], final=True)
    if not nc.is_finalized():
        nc.finalize()
    return nc


def _get_nc(nparts):
    if nparts not in _nc_cache:
        _nc_cache[nparts] = _build_nc(nparts)
    return _nc_cache[nparts]


def _segsum(B, col_idx, n_cols):
    """out[j, n] = sum_{e: col_idx[e]==n} B[j, e]  (B float32 [J, E])."""
    E = B.shape[1]
    perm = np.argsort(col_idx, kind="stable")
    starts = np.searchsorted(col_idx[perm], np.arange(n_cols))
    out = np.add.reduceat(B[:, perm], np.minimum(starts, E - 1), axis=1)
    counts = np.bincount(col_idx, minlength=n_cols)
    out[:, counts == 0] = 0.0
    return out


def _host_terms(na, L, idx, kept):
    """Build const [N_CMP], W [N_CMP, N_NODES], Q [N_CMP, N_EDGES]."""
    const = (L == 0.0).sum(axis=1).astype(np.float64)
    W = np.zeros((N_CMP, N_NODES), np.float32)
    Q = np.zeros((N_CMP, N_EDGES), np.float32)
    for v in kept:
        t = _CODE2TEMP[v]
        c, l0, l1, q = _COEF[t]
        Bv = (L == float(v)).astype(np.float32)
        if c:
            const = const + c * Bv.sum(axis=1, dtype=np.float64)
        Q += q * Bv
        for k, lk in ((0, l0), (1, l1)):
            if lk:
                W += lk * _segsum(Bv, idx[:, k], N_NODES)
    return const.astype(np.float32), W, Q


def _split_w(W):
    """Exact fp8-e4m3 split: W = sum(parts), each part integer |w|<=16."""
    parts = []
    R = W.copy()
    while np.any(R):
        part = np.clip(R, -16.0, 16.0)
        parts.append(part)
        R = R - part
    return parts


def _prep_inputs(na, L, idx, kept):
    nau8 = na.astype(np.uint8)
    const, W, Q = _host_terms(nau8, L, idx, kept)
    wparts = _split_w(W)
    nparts = len(wparts)
    nchunk = ECHUNKS + NCHUNKS * nparts
    nblks = nchunk // ABLK

    AND = (nau8[:, idx[:, 0]] & nau8[:, idx[:, 1]]).astype(np.float32)
    X = np.concatenate([AND] + [nau8.astype(np.float32)] * nparts, axis=1)
    R = np.concatenate([Q] + wparts, axis=1)

    xt_pgs = []
    for pg in range(PGROUPS):
        xs = X[pg * P:(pg + 1) * P]
        xt_pgs.append(np.ascontiguousarray(
            xs.reshape(P, nchunk, 128).transpose(2, 1, 0)).astype(
                NP_FP8).reshape(128, nchunk * P))
    rt_cgs = []
    for cg in range(CGROUPS):
        rs = R[cg * C:(cg + 1) * C]
        rt_cgs.append(np.ascontiguousarray(
            rs.reshape(NTILES, 512, nblks, ABLK, 128)
            .transpose(0, 2, 4, 3, 1)).astype(
                NP_FP8).reshape(NTILES, nblks, 128, ABLK * 512))

    in_maps = []
    for pg in range(PGROUPS):
        for cg in range(CGROUPS):
            in_maps.append({"XT": xt_pgs[pg], "RT": rt_cgs[cg]})
    return in_maps, const, nparts


def _kept_vals(edge_type_filter):
    seen = []
    for v in np.asarray(edge_type_filter).ravel().tolist():
        v = int(v)
        if v in _CODE2TEMP and v not in seen:
            seen.append(v)
    return seen


def kernel(node_activations, learned_edge_states, edge_endnode_idx,
           edge_type_filter, _trace=False, _tmpdir=None):
    na = np.asarray(node_activations)
    L = np.asarray(learned_edge_states, dtype=np.float32)
    idx = np.asarray(edge_endnode_idx)
    kept = _kept_vals(edge_type_filter)
    if len(kept) == 0:
        null_count = (L == 0.0).sum(axis=1).astype(np.float32)
        en = np.broadcast_to(null_count[None, :], (N_PTS, N_CMP)).copy()
        return en - en.min()

    in_maps, const, nparts = _prep_inputs(na, L, idx, kept)
    nc = _get_nc(nparts)
    res = run_bass_kernel_spmd(nc, in_maps, core_ids=list(range(8)),
                               trace=_trace, tmpdir=_tmpdir)
    out = np.empty((N_PTS, N_CMP), dtype=np.float32)
    for ci in range(8):
        pg, cg = ci // CGROUPS, ci % CGROUPS
        out[pg * P:(pg + 1) * P, cg * C:(cg + 1) * C] = res.results[ci]["en"]
    out += const[None, :]
    out -= out.min()
    if _trace:
        kernel._last_results = res
    return out
